# revision 2
# baseline (speedup 1.0000x reference)
"""Trainium2 Bass kernel for multi-head causal self-attention.

Problem (hardcoded): B=4, T=2048, C=1024, H=16 heads, D=64, fp32.
  qkv = x @ W_t + b; split into q,k,v; causal softmax(q k^T / sqrt(D)) @ v.

Sharding over 8 NeuronCores: core c handles batch b = c//2 and head group
hg = c%2 (8 heads). No cross-device communication.

Per-core layout strategy:
  - inputs DMA'd as xT [C, T] (host-transposed), W slices [C, 512].
  - QT/KT computed d-major [512, T] (fp32r), V natural [T, 512] (fp16).
  - scores computed transposed: ST[k, q] = KT^T-block @ QT (fp32r matmuls,
    full PE rate at moving dim >= 256), exp on ScalarE straight out of PSUM
    (one op per multi-bank group), causal diagonal masked by a triangle
    multiply on VectorE, P stored fp16.
  - AV matmul col-tiled: V_h at array cols 0-63 and an all-ones block at
    cols 64-127, so PSUM rows 64:128 accumulate the softmax denominator
    broadcast across 64 partitions for free.
  - normalize: copy denom to SBUF, reciprocal_approx_fast, multiply.
  - output written as YT [512, T] per core; host transposes/gathers.
"""
import sys
import types
from contextlib import ExitStack

import numpy as np

import concourse.bass as bass
import concourse.tile as tile
import concourse.mybir as mybir
from concourse import bacc
from concourse import bass_utils

B, T, C = 4, 2048, 1024
H = 16
D = 64
N_CORES = 8
HEADS_PER_CORE = 8          # tensor-parallel over 2 head groups
HG_COLS = HEADS_PER_CORE * D  # 512
N_TC = T // 512             # 4 t-chunks (q-chunks)
N_CC = C // 128             # 8 contraction chunks
SCALE = float(1.0 / np.sqrt(D))

F32 = mybir.dt.float32
F32R = mybir.dt.float32r
F16 = mybir.dt.float16

_NC_CACHE = {}


def _install_ntff_hook():
    if "antenv.axon_hooks" in sys.modules:
        return
    try:
        from trn_agent_boot.trn_boot import _ntff_profile_via_ctypes
    except ImportError:
        return
    mod = types.ModuleType("antenv.axon_hooks")
    _hook = [None]
    mod.set_axon_ntff_profile_hook = lambda h: _hook.__setitem__(0, h)
    mod.get_axon_ntff_profile_hook = lambda: _hook[0]
    sys.modules["antenv.axon_hooks"] = mod
    hook = _ntff_profile_via_ctypes("/opt/axon/libaxon_pjrt.so")
    if hook is not None:
        mod.set_axon_ntff_profile_hook(hook)


def _build_nc():
    nc = bacc.Bacc("TRN2", target_bir_lowering=False, debug=False,
                   num_devices=N_CORES)

    xt_ap = nc.dram_tensor("xt", [C, T], F32R, kind="ExternalInput").ap()
    wq_ap = nc.dram_tensor("wq", [C, HG_COLS], F32R, kind="ExternalInput").ap()
    wk_ap = nc.dram_tensor("wk", [C, HG_COLS], F32R, kind="ExternalInput").ap()
    wv_ap = nc.dram_tensor("wv", [C, HG_COLS], F32R, kind="ExternalInput").ap()
    bq_ap = nc.dram_tensor("bq", [128, 4], F32, kind="ExternalInput").ap()
    bk_ap = nc.dram_tensor("bk", [128, 4], F32, kind="ExternalInput").ap()
    bv_ap = nc.dram_tensor("bv", [128, HG_COLS], F32, kind="ExternalInput").ap()
    tri_ap = nc.dram_tensor("tri", [128, 128], F16, kind="ExternalInput").ap()
    out_ap = nc.dram_tensor("out", [HG_COLS, T], F32, kind="ExternalOutput").ap()

    with tile.TileContext(nc) as tc, ExitStack() as ctx:
        consts = ctx.enter_context(tc.tile_pool(name="consts", bufs=1))
        xt_pool = ctx.enter_context(tc.tile_pool(name="xt", bufs=2))
        qkv_pool = ctx.enter_context(tc.tile_pool(name="qkv", bufs=1))
        ex_pool = ctx.enter_context(tc.tile_pool(name="ex", bufs=4))
        nrm_pool = ctx.enter_context(tc.tile_pool(name="nrm", bufs=4))
        ps_pool = ctx.enter_context(tc.tile_pool(name="ps", bufs=2, space="PSUM"))
        y_pool = ctx.enter_context(tc.tile_pool(name="yps", bufs=2, space="PSUM"))

        wq_sb = consts.tile([128, N_CC, HG_COLS], F32R, tag="wq")
        wk_sb = consts.tile([128, N_CC, HG_COLS], F32R, tag="wk")
        wv_sb = consts.tile([128, N_CC, HG_COLS], F32R, tag="wv")
        bq_sb = consts.tile([128, 4], F32, tag="bq")
        bk_sb = consts.tile([128, 4], F32, tag="bk")
        bv_sb = consts.tile([128, HG_COLS], F32, tag="bv")
        tri_sb = consts.tile([128, 128], F16, tag="tri")
        ones_sb = consts.tile([128, 64], F16, tag="ones")

        nc.sync.dma_start(out=wq_sb, in_=wq_ap.rearrange("(c p) j -> p c j", p=128))
        nc.sync.dma_start(out=wk_sb, in_=wk_ap.rearrange("(c p) j -> p c j", p=128))
        nc.sync.dma_start(out=wv_sb, in_=wv_ap.rearrange("(c p) j -> p c j", p=128))
        nc.sync.dma_start(out=bq_sb, in_=bq_ap)
        nc.sync.dma_start(out=bk_sb, in_=bk_ap)
        nc.sync.dma_start(out=bv_sb, in_=bv_ap)
        nc.sync.dma_start(out=tri_sb, in_=tri_ap)
        nc.vector.memset(ones_sb, 1.0)

        # persistent activations
        qt_sb = qkv_pool.tile([128, 4, T], F32R, tag="qt")   # [d-in-block, dblk, t]
        kt_sb = qkv_pool.tile([128, 4, T], F32R, tag="kt")
        v_sb = qkv_pool.tile([128, HEADS_PER_CORE, T // 128, D], F16, tag="v")

        xt_re = xt_ap.rearrange("(c p) t -> p c t", p=128)

        def emit_qkv(tcn):
            t0 = tcn * 512
            xt = xt_pool.tile([128, N_CC, 512], F32R, tag="xt")
            nc.sync.dma_start(out=xt, in_=xt_re[:, :, t0:t0 + 512])
            for which, w_sb, dst, b_sb in (("q", wq_sb, qt_sb, bq_sb),
                                           ("k", wk_sb, kt_sb, bk_sb)):
                for db in range(4):
                    pq = ps_pool.tile([128, 512], F32, tag="stg")
                    for cc in range(N_CC):
                        nc.tensor.matmul(
                            pq,
                            w_sb[:, cc, db * 128:(db + 1) * 128],
                            xt[:, cc, :],
                            start=(cc == 0),
                            stop=(cc == N_CC - 1),
                        )
                    nc.scalar.activation(
                        dst[:, db, t0:t0 + 512], pq,
                        mybir.ActivationFunctionType.Identity,
                        bias=b_sb[:, db:db + 1], scale=1.0,
                    )
            for tt in range(4):
                gt = tcn * 4 + tt  # global t-tile
                pv = ps_pool.tile([128, 512], F32, tag="stg")
                for cc in range(N_CC):
                    nc.tensor.matmul(
                        pv,
                        xt[:, cc, tt * 128:(tt + 1) * 128],
                        wv_sb[:, cc, :],
                        start=(cc == 0),
                        stop=(cc == N_CC - 1),
                    )
                nc.vector.tensor_add(
                    v_sb[:, :, gt, :],
                    pv.rearrange("p (h d) -> p h d", h=HEADS_PER_CORE),
                    bv_sb.rearrange("p (h d) -> p h d", h=HEADS_PER_CORE),
                )

        def emit_attn(qi):
            q0 = qi * 512
            nkt = 4 * qi + 4
            for pr in range(4):
                y_ps = {}
                for hl, base in ((0, 0), (1, 64)):
                    y_ps[hl] = y_pool.tile([128, 512], F32, tag="y", name=f"y{hl}")
                for g in range(0, nkt, 3):
                    kts = list(range(g, min(g + 3, nkt)))
                    for hl, base in ((0, 0), (1, 64)):
                        stg = ps_pool.tile([128, 3, 512], F32, tag="stg")
                        for idx, kt in enumerate(kts):
                            j = kt - 4 * qi
                            s = 0 if j < 0 else min(128 * j, 256)
                            nc.tensor.matmul(
                                stg[:, idx, s:512],
                                kt_sb[base:base + 64, pr, kt * 128:(kt + 1) * 128],
                                qt_sb[base:base + 64, pr, q0 + s:q0 + 512],
                                start=True, stop=True,
                                tile_position=(base, 0),
                            )
                        ex = ex_pool.tile([128, 3, 512], F16, tag="ex")
                        nc.scalar.activation(
                            ex[:, 0:len(kts), :].rearrange("p a b -> p (a b)"),
                            stg[:, 0:len(kts), :].rearrange("p a b -> p (a b)"),
                            mybir.ActivationFunctionType.Exp,
                            scale=SCALE,
                        )
                        for idx, kt in enumerate(kts):
                            j = kt - 4 * qi
                            if j >= 0:
                                blk = ex[:, idx, 128 * j:128 * (j + 1)]
                                nc.vector.tensor_mul(blk, blk, tri_sb)
                        h = 2 * pr + hl
                        for idx, kt in enumerate(kts):
                            j = kt - 4 * qi
                            av_s = 0 if j < 0 else 128 * j
                            nc.tensor.matmul(
                                y_ps[hl][0:64, av_s:512],
                                v_sb[:, h, kt, :],
                                ex[:, idx, av_s:512],
                                start=(kt == 0), stop=(kt == nkt - 1),
                                tile_position=(0, 0),
                                skip_group_check=True,
                            )
                            nc.tensor.matmul(
                                y_ps[hl][64:128, av_s:512],
                                ones_sb,
                                ex[:, idx, av_s:512],
                                start=(kt == 0), stop=(kt == nkt - 1),
                                tile_position=(0, 64),
                                skip_group_check=True,
                            )
                for hl in (0, 1):
                    h = 2 * pr + hl
                    den = nrm_pool.tile([64, 512], F32, tag="den")
                    nc.vector.tensor_copy(den, y_ps[hl][64:128, :])
                    rec = nrm_pool.tile([64, 512], F32, tag="rec")
                    nc.vector.reciprocal_approx_fast(out=rec, in_=den)
                    yf = nrm_pool.tile([64, 512], F32, tag="yf")
                    nc.vector.tensor_mul(yf, y_ps[hl][0:64, :], rec)
                    nc.sync.dma_start(
                        out=out_ap[h * D:(h + 1) * D, q0:q0 + 512], in_=yf)

        for tcn in range(N_TC):
            emit_qkv(tcn)
            emit_attn(tcn)

    nc.compile()
    return nc


def _get_nc():
    if "nc" not in _NC_CACHE:
        _NC_CACHE["nc"] = _build_nc()
    return _NC_CACHE["nc"]


def _make_in_maps(x, W_t, b):
    x = np.asarray(x, dtype=np.float32)
    W_t = np.asarray(W_t, dtype=np.float32)
    b = np.asarray(b, dtype=np.float32)
    tri = np.triu(np.ones((128, 128), dtype=np.float16))  # [k, q]: valid k<=q
    in_maps = []
    for core in range(N_CORES):
        bb, hg = core // 2, core % 2
        cs = hg * HG_COLS
        in_maps.append({
            "xt": np.ascontiguousarray(x[bb].T),
            "wq": np.ascontiguousarray(W_t[:, cs:cs + HG_COLS]),
            "wk": np.ascontiguousarray(W_t[:, C + cs:C + cs + HG_COLS]),
            "wv": np.ascontiguousarray(W_t[:, 2 * C + cs:2 * C + cs + HG_COLS]),
            "bq": np.ascontiguousarray(b[cs:cs + HG_COLS].reshape(4, 128).T),
            "bk": np.ascontiguousarray(b[C + cs:C + cs + HG_COLS].reshape(4, 128).T),
            "bv": np.ascontiguousarray(
                np.broadcast_to(b[2 * C + cs:2 * C + cs + HG_COLS], (128, HG_COLS))),
            "tri": tri,
        })
    return in_maps


def _gather(results):
    y = np.empty((B, T, C), dtype=np.float32)
    for core in range(N_CORES):
        bb, hg = core // 2, core % 2
        y[bb, :, hg * HG_COLS:(hg + 1) * HG_COLS] = results[core]["out"].T
    return y


def _run(x, W_t, b, trace=False):
    nc = _get_nc()
    in_maps = _make_in_maps(x, W_t, b)
    if trace:
        _install_ntff_hook()
    res = bass_utils.run_bass_kernel_spmd(
        nc, in_maps, core_ids=list(range(N_CORES)), trace=trace)
    return _gather(res.results), res.exec_time_ns


def kernel(x, W_t, b):
    y, _ = _run(x, W_t, b, trace=False)
    return y


def kernel_traced(x, W_t, b):
    """Returns (y, hw_exec_time_ns). Used by test.py for profiling."""
    return _run(x, W_t, b, trace=True)


# revision 3
# speedup vs baseline: 1.3158x; 1.3158x over previous
"""Trainium2 Bass kernel for multi-head causal self-attention.

Problem (hardcoded): B=4, T=2048, C=1024, H=16 heads, D=64, fp32.
  qkv = x @ W_t + b; split into q,k,v; causal softmax(q k^T / sqrt(D)) @ v.

Sharding over 8 NeuronCores: core c handles batch b = c//2 and head group
hg = c%2 (8 heads). No cross-device communication.

Per-core layout strategy:
  - inputs DMA'd as xT [C, T] (host-transposed), W slices [C, 512].
  - QT/KT computed d-major [512, T] (fp32r), V natural [T, 512] (fp16).
  - scores computed transposed: ST[k, q] = KT^T-block @ QT (fp32r matmuls,
    full PE rate at moving dim >= 256), exp on ScalarE straight out of PSUM
    (one op per multi-bank group), causal diagonal masked by a triangle
    multiply on VectorE, P stored fp16.
  - AV matmul col-tiled: V_h at array cols 0-63 and an all-ones block at
    cols 64-127, so PSUM rows 64:128 accumulate the softmax denominator
    broadcast across 64 partitions for free.
  - normalize: copy denom to SBUF, reciprocal_approx_fast, multiply.
  - output written as YT [512, T] per core; host transposes/gathers.
"""
import sys
import types
from contextlib import ExitStack

import numpy as np

import concourse.bass as bass
import concourse.tile as tile
import concourse.mybir as mybir
from concourse import bacc
from concourse import bass_utils

B, T, C = 4, 2048, 1024
H = 16
D = 64
N_CORES = 8
HEADS_PER_CORE = 8          # tensor-parallel over 2 head groups
HG_COLS = HEADS_PER_CORE * D  # 512
N_TC = T // 512             # 4 t-chunks (q-chunks)
N_CC = C // 128             # 8 contraction chunks
SCALE = float(1.0 / np.sqrt(D))

F32 = mybir.dt.float32
F32R = mybir.dt.float32r
F16 = mybir.dt.float16

_NC_CACHE = {}


def _install_ntff_hook():
    if "antenv.axon_hooks" in sys.modules:
        return
    try:
        from trn_agent_boot.trn_boot import _ntff_profile_via_ctypes
    except ImportError:
        return
    mod = types.ModuleType("antenv.axon_hooks")
    _hook = [None]
    mod.set_axon_ntff_profile_hook = lambda h: _hook.__setitem__(0, h)
    mod.get_axon_ntff_profile_hook = lambda: _hook[0]
    sys.modules["antenv.axon_hooks"] = mod
    hook = _ntff_profile_via_ctypes("/opt/axon/libaxon_pjrt.so")
    if hook is not None:
        mod.set_axon_ntff_profile_hook(hook)


def _build_nc():
    nc = bacc.Bacc("TRN2", target_bir_lowering=False, debug=False,
                   num_devices=N_CORES)

    xt_ap = nc.dram_tensor("xt", [C, T], F32R, kind="ExternalInput").ap()
    wq_ap = nc.dram_tensor("wq", [C, HG_COLS], F32R, kind="ExternalInput").ap()
    wk_ap = nc.dram_tensor("wk", [C, HG_COLS], F32R, kind="ExternalInput").ap()
    wv_ap = nc.dram_tensor("wv", [C, HG_COLS], F32R, kind="ExternalInput").ap()
    bq_ap = nc.dram_tensor("bq", [128, 4], F32, kind="ExternalInput").ap()
    bk_ap = nc.dram_tensor("bk", [128, 4], F32, kind="ExternalInput").ap()
    bv_ap = nc.dram_tensor("bv", [128, HG_COLS], F32, kind="ExternalInput").ap()
    tri_ap = nc.dram_tensor("tri", [128, 128], F16, kind="ExternalInput").ap()
    out_ap = nc.dram_tensor("out", [HG_COLS, T], F32, kind="ExternalOutput").ap()

    with tile.TileContext(nc) as tc, ExitStack() as ctx:
        consts = ctx.enter_context(tc.tile_pool(name="consts", bufs=1))
        xt_pool = ctx.enter_context(tc.tile_pool(name="xt", bufs=2))
        qkv_pool = ctx.enter_context(tc.tile_pool(name="qkv", bufs=1))
        ex_pool = ctx.enter_context(tc.tile_pool(name="ex", bufs=6))
        nrm_pool = ctx.enter_context(tc.tile_pool(name="nrm", bufs=4))
        ps_pool = ctx.enter_context(tc.tile_pool(name="ps", bufs=2, space="PSUM"))
        y_pool = ctx.enter_context(tc.tile_pool(name="yps", bufs=2, space="PSUM"))

        wq_sb = consts.tile([128, N_CC, HG_COLS], F32R, tag="wq")
        wk_sb = consts.tile([128, N_CC, HG_COLS], F32R, tag="wk")
        wv_sb = consts.tile([128, N_CC, HG_COLS], F32R, tag="wv")
        bq_sb = consts.tile([128, 4], F32, tag="bq")
        bk_sb = consts.tile([128, 4], F32, tag="bk")
        bv_sb = consts.tile([128, HG_COLS], F32, tag="bv")
        tri_sb = consts.tile([128, 128], F16, tag="tri")

        nc.sync.dma_start(out=wq_sb, in_=wq_ap.rearrange("(c p) j -> p c j", p=128))
        nc.sync.dma_start(out=wk_sb, in_=wk_ap.rearrange("(c p) j -> p c j", p=128))
        nc.sync.dma_start(out=wv_sb, in_=wv_ap.rearrange("(c p) j -> p c j", p=128))
        nc.sync.dma_start(out=bq_sb, in_=bq_ap)
        nc.sync.dma_start(out=bk_sb, in_=bk_ap)
        nc.sync.dma_start(out=bv_sb, in_=bv_ap)
        nc.sync.dma_start(out=tri_sb, in_=tri_ap)

        # persistent activations
        qt_sb = qkv_pool.tile([128, 4, T], F16, tag="qt")   # [d-in-block, dblk, t]
        kt_sb = qkv_pool.tile([128, 4, T], F16, tag="kt")
        # V with a baked-in all-ones block at cols 64:128 (denominator trick)
        v_sb = qkv_pool.tile([128, HEADS_PER_CORE, T // 128, 2 * D], F16, tag="v")
        nc.vector.memset(v_sb[:, :, :, D:2 * D], 1.0)

        xt_re = xt_ap.rearrange("(c p) t -> p c t", p=128)

        def emit_qkv(tcn):
            t0 = tcn * 512
            xt = xt_pool.tile([128, N_CC, 512], F32R, tag="xt")
            nc.sync.dma_start(out=xt, in_=xt_re[:, :, t0:t0 + 512])
            for which, w_sb, dst, b_sb in (("q", wq_sb, qt_sb, bq_sb),
                                           ("k", wk_sb, kt_sb, bk_sb)):
                for db in range(4):
                    pq = ps_pool.tile([128, 512], F32, tag="stg")
                    for cc in range(N_CC):
                        nc.tensor.matmul(
                            pq,
                            w_sb[:, cc, db * 128:(db + 1) * 128],
                            xt[:, cc, :],
                            start=(cc == 0),
                            stop=(cc == N_CC - 1),
                        )
                    nc.scalar.activation(
                        dst[:, db, t0:t0 + 512], pq,
                        mybir.ActivationFunctionType.Identity,
                        bias=b_sb[:, db:db + 1], scale=1.0,
                    )
            for tt in range(4):
                gt = tcn * 4 + tt  # global t-tile
                pv = ps_pool.tile([128, 512], F32, tag="stg")
                for cc in range(N_CC):
                    nc.tensor.matmul(
                        pv,
                        xt[:, cc, tt * 128:(tt + 1) * 128],
                        wv_sb[:, cc, :],
                        start=(cc == 0),
                        stop=(cc == N_CC - 1),
                    )
                nc.vector.tensor_add(
                    v_sb[:, :, gt, 0:D],
                    pv.rearrange("p (h d) -> p h d", h=HEADS_PER_CORE),
                    bv_sb.rearrange("p (h d) -> p h d", h=HEADS_PER_CORE),
                )

        def emit_attn(qi):
            q0 = qi * 512
            nkt = 4 * qi + 4
            groups = [list(range(g, min(g + 3, nkt))) for g in range(0, nkt, 3)]
            for pr in range(4):
                y_ps = {}
                for hl in (0, 1):
                    y_ps[hl] = y_pool.tile([128, 512], F32, tag="y", name=f"y{hl}")

                def emit_av(kts, exs):
                    for hl in (0, 1):
                        h = 2 * pr + hl
                        for idx, kt in enumerate(kts):
                            j = kt - 4 * qi
                            av_s = 0 if j < 0 else 128 * j
                            nc.tensor.matmul(
                                y_ps[hl][:, av_s:512],
                                v_sb[:, h, kt, :],
                                exs[hl][:, idx, av_s:512],
                                start=(kt == 0), stop=(kt == nkt - 1),
                                skip_group_check=True,
                            )

                prev = None
                for kts in groups:
                    stg = {}
                    for hl in (0, 1):
                        stg[hl] = ps_pool.tile([128, 3, 512], F32, tag="stg",
                                               name=f"stg{hl}")
                    for idx, kt in enumerate(kts):
                        j = kt - 4 * qi
                        s = 0 if j < 0 else min(128 * j, 256)
                        for hl, base in ((0, 0), (1, 64)):
                            nc.tensor.matmul(
                                stg[hl][:, idx, s:512],
                                kt_sb[base:base + 64, pr, kt * 128:(kt + 1) * 128],
                                qt_sb[base:base + 64, pr, q0 + s:q0 + 512],
                                start=True, stop=True,
                                tile_position=(base, 0),
                            )
                    exs = {}
                    for hl in (0, 1):
                        ex = ex_pool.tile([128, 3, 512], F16, tag="ex",
                                          name=f"ex{hl}")
                        nc.scalar.activation(
                            ex[:, 0:len(kts), :].rearrange("p a b -> p (a b)"),
                            stg[hl][:, 0:len(kts), :].rearrange("p a b -> p (a b)"),
                            mybir.ActivationFunctionType.Exp,
                            scale=SCALE,
                        )
                        for idx, kt in enumerate(kts):
                            j = kt - 4 * qi
                            if j >= 0:
                                blk = ex[:, idx, 128 * j:128 * (j + 1)]
                                nc.vector.tensor_mul(blk, blk, tri_sb)
                        exs[hl] = ex
                    if prev is not None:
                        emit_av(*prev)
                    prev = (kts, exs)
                emit_av(*prev)

                for hl in (0, 1):
                    h = 2 * pr + hl
                    den = nrm_pool.tile([64, 512], F32, tag="den")
                    nc.vector.tensor_copy(den, y_ps[hl][64:128, :])
                    rec = nrm_pool.tile([64, 512], F32, tag="rec")
                    nc.vector.reciprocal_approx_fast(out=rec, in_=den)
                    yf = nrm_pool.tile([64, 512], F32, tag="yf")
                    nc.vector.tensor_mul(yf, y_ps[hl][0:64, :], rec)
                    nc.sync.dma_start(
                        out=out_ap[h * D:(h + 1) * D, q0:q0 + 512], in_=yf)

        for tcn in range(N_TC):
            emit_qkv(tcn)
            emit_attn(tcn)

    nc.compile()
    return nc


def _get_nc():
    if "nc" not in _NC_CACHE:
        _NC_CACHE["nc"] = _build_nc()
    return _NC_CACHE["nc"]


def _make_in_maps(x, W_t, b):
    x = np.asarray(x, dtype=np.float32)
    W_t = np.asarray(W_t, dtype=np.float32)
    b = np.asarray(b, dtype=np.float32)
    tri = np.triu(np.ones((128, 128), dtype=np.float16))  # [k, q]: valid k<=q
    in_maps = []
    for core in range(N_CORES):
        bb, hg = core // 2, core % 2
        cs = hg * HG_COLS
        in_maps.append({
            "xt": np.ascontiguousarray(x[bb].T),
            "wq": np.ascontiguousarray(W_t[:, cs:cs + HG_COLS]),
            "wk": np.ascontiguousarray(W_t[:, C + cs:C + cs + HG_COLS]),
            "wv": np.ascontiguousarray(W_t[:, 2 * C + cs:2 * C + cs + HG_COLS]),
            "bq": np.ascontiguousarray(b[cs:cs + HG_COLS].reshape(4, 128).T),
            "bk": np.ascontiguousarray(b[C + cs:C + cs + HG_COLS].reshape(4, 128).T),
            "bv": np.ascontiguousarray(
                np.broadcast_to(b[2 * C + cs:2 * C + cs + HG_COLS], (128, HG_COLS))),
            "tri": tri,
        })
    return in_maps


def _gather(results):
    y = np.empty((B, T, C), dtype=np.float32)
    for core in range(N_CORES):
        bb, hg = core // 2, core % 2
        y[bb, :, hg * HG_COLS:(hg + 1) * HG_COLS] = results[core]["out"].T
    return y


def _run(x, W_t, b, trace=False):
    nc = _get_nc()
    in_maps = _make_in_maps(x, W_t, b)
    if trace:
        _install_ntff_hook()
    res = bass_utils.run_bass_kernel_spmd(
        nc, in_maps, core_ids=list(range(N_CORES)), trace=trace)
    return _gather(res.results), res.exec_time_ns


def kernel(x, W_t, b):
    y, _ = _run(x, W_t, b, trace=False)
    return y


def kernel_traced(x, W_t, b):
    """Returns (y, hw_exec_time_ns). Used by test.py for profiling."""
    return _run(x, W_t, b, trace=True)
